# revision 2
# baseline (speedup 1.0000x reference)
"""Trainium2 Bass kernel for the constrained-CKY hinge loss problem.

Reference computation (fp32):
  - Two max-plus CKY DPs over a triangular chart (LENGTH=128, BATCH=256):
    one from a zero chart, one from a chart with +1000 bonuses at 8
    constraint cells per example.
  - Loss = masked mean of hinge(MARGIN + pred - constr).

Sharding: pure data parallel over (chart-type x batch-quarter):
  core c in 0..7 runs chart type c//4 (0=plain, 1=constrained) for batch
  slice (c%4)*64 : (c%4+1)*64.  64 batch rows live on 64 SBUF partitions;
  the whole DP for a row runs on its partition.

Chart layout per row: C[row][pos*128 + w] = cell(width=w, start=pos).
  At level l (L=128-l positions, N=l split points):
    ls[pos,n] = C[pos*128 + n]                  (strides pos:128, n:1)
    rs[pos,n] = C[(pos+n+1)*128 + (l-1-n)]
              = C[pos*128 + n*127 + (127+l)]    (strides pos:128, n:127)
  best[pos] = max_n(ls + rs + x);  C[pos*128 + l] = best + init_col.
"""

import sys

if "/opt/trn_rl_repo" not in sys.path:
    sys.path.insert(0, "/opt/trn_rl_repo")

import numpy as np

LENGTH = 128
BATCH = 256
MARGIN = 1.0
BONUS = 1000.0
NCELLS = LENGTH * (LENGTH + 1) // 2  # 8256
NCONSTR = 8
N_CORES = 8
ROWS = 64  # batch rows per core
CSTRIDE = 128  # free-dim stride between consecutive `pos` in the chart tile


def _offsets():
    off = np.zeros(LENGTH, dtype=np.int64)
    for lvl in range(1, LENGTH):
        off[lvl] = off[lvl - 1] + (LENGTH - (lvl - 1))
    return off


OFF = _offsets()


def _score_offsets():
    soff, acc = {}, 0
    for level in range(1, LENGTH):
        soff[level] = acc
        acc += (LENGTH - level) * level
    return soff, acc


SOFF, TOTAL = _score_offsets()  # TOTAL = 349504

_COMPILED = None


def _build_program(repeat=1):
    from concourse import bacc, bass, mybir
    from concourse import tile

    nc = bacc.Bacc("TRN2", target_bir_lowering=False, debug=False,
                   num_devices=N_CORES)
    scores_ext = nc.dram_tensor("scores", [ROWS, TOTAL], mybir.dt.float32,
                                kind="ExternalInput")
    roots_ext = nc.dram_tensor("roots", [ROWS, 1], mybir.dt.float32,
                               kind="ExternalOutput")

    f32 = mybir.dt.float32
    ADD = mybir.AluOpType.add
    MAX = mybir.AluOpType.max

    with tile.TileContext(nc) as tc:
        with (
            tc.tile_pool(name="persist", bufs=1) as persist,
            tc.tile_pool(name="xs", bufs=6) as xpool,
            tc.tile_pool(name="tmp", bufs=1) as tpool,
            tc.tile_pool(name="tmp2", bufs=1) as upool,
        ):
            C = persist.tile([ROWS, LENGTH * CSTRIDE], f32)

            c_full = C[:]
            c_part = c_full.ap[0]  # [partition_stride, ROWS]

            def c_view(offset, dims):
                return bass.AP(tensor=c_full.tensor,
                               offset=c_full.offset + offset,
                               ap=[c_part] + dims)

            for _rep in range(repeat):
                # level 0 = 0 everywhere (bonuses are folded into scores
                # on the host; constraint cells always have width >= 1).
                nc.vector.memset(C[:], 0.0)

                for l in range(1, LENGTH):
                    L = LENGTH - l
                    N = l
                    E = L * N

                    x_t = xpool.tile([ROWS, E], f32, tag="x")
                    nc.sync.dma_start(out=x_t[:],
                                      in_=scores_ext[:, SOFF[l]:SOFF[l] + E])

                    ls = c_view(0, [[CSTRIDE, L], [1, N]])
                    rs = c_view(127 + l, [[CSTRIDE, L], [127, N]])

                    t_t = tpool.tile([ROWS, E], f32, tag="t")
                    tf = t_t[:]
                    t3 = bass.AP(tensor=tf.tensor, offset=tf.offset,
                                 ap=[tf.ap[0], [N, L], [1, N]])
                    nc.vector.tensor_tensor(out=t3, in0=ls, in1=rs, op=ADD)

                    u_t = upool.tile([ROWS, E], f32, tag="u")
                    nc.vector.tensor_tensor(out=u_t[:], in0=t_t[:],
                                            in1=x_t[:], op=ADD)
                    uf = u_t[:]
                    u3 = bass.AP(tensor=uf.tensor, offset=uf.offset,
                                 ap=[uf.ap[0], [N, L], [1, N]])

                    # best[pos] written straight into the chart column
                    nc.vector.tensor_reduce(out=c_view(l, [[CSTRIDE, L]]),
                                            in_=u3,
                                            axis=mybir.AxisListType.X, op=MAX)

            # root = cell(width=127, pos=0) = C[127]
            nc.sync.dma_start(out=roots_ext[:], in_=C[:, 127:128])

    nc.compile()
    return nc


def _get_compiled():
    global _COMPILED
    if _COMPILED is None:
        _COMPILED = _build_program()
    return _COMPILED


def make_inmaps(np_inputs):
    scores = np.asarray(np_inputs["scores"], dtype=np.float32)
    constraint_pos = np.asarray(np_inputs["constraint_pos"], dtype=np.int32)
    B = scores.shape[0]
    assert B == BATCH and scores.shape[1] == TOTAL

    # Constrained-chart scores: fold the +BONUS of cell (w, p) into every
    # split entry x[l=w][p, n] of that cell (exact up to 1 ulp vs adding it
    # after the reduce).  Set semantics (duplicate constraints count once),
    # matching `chart.at[rows, pos].set(BONUS)` in the reference.
    folded = scores.copy()
    bonus = np.float32(BONUS)
    for b in range(B):
        for f in set(int(v) for v in constraint_pos[b]):
            w = int(np.searchsorted(OFF, f, side="right")) - 1
            p = f - int(OFF[w])
            s = SOFF[w] + p * w
            folded[b, s:s + w] += bonus

    in_maps = []
    for c in range(N_CORES):
        q = c % 4
        sl = slice(q * ROWS, (q + 1) * ROWS)
        src = scores if c < 4 else folded
        in_maps.append({"scores": np.ascontiguousarray(src[sl])})
    return in_maps


def kernel(scores, constraint_pos, trace=False):
    from concourse.bass_utils import run_bass_kernel_spmd

    in_maps = make_inmaps({"scores": scores, "constraint_pos": constraint_pos})

    nc = _get_compiled()
    res = run_bass_kernel_spmd(nc, in_maps, list(range(N_CORES)), trace=trace)

    pred = np.concatenate([res.results[q]["roots"][:, 0] for q in range(4)])
    constr_root = np.concatenate(
        [res.results[4 + q]["roots"][:, 0] for q in range(4)])

    pred = pred.astype(np.float32)
    constr = (constr_root - np.float32(BONUS * NCONSTR)).astype(np.float32)

    mask = (np.abs(pred - constr) >= np.float32(0.001)).astype(np.float32)
    hinge = np.maximum(np.float32(MARGIN) + pred - constr,
                       np.float32(0.0)) * mask
    msum = mask.sum(dtype=np.float32)
    hsum = hinge.sum(dtype=np.float32)
    if msum > np.float32(0.1):
        out = hsum / np.maximum(msum, np.float32(1.0))
    else:
        out = hsum
    result = np.asarray(out, dtype=np.float32)
    if trace:
        return result, res
    return result



# revision 7
# speedup vs baseline: 3.7957x; 3.7957x over previous
"""Trainium2 Bass kernel for the constrained-CKY hinge loss problem.

Reference computation (fp32):
  - Two max-plus CKY DPs over a triangular chart (LENGTH=128, BATCH=256):
    one from a zero chart, one from a chart with +1000 bonuses at 8
    constraint cells per example.
  - Loss = masked mean of hinge(MARGIN + pred - constr).

Sharding: (chart-type x batch-quarter) across cores, position-parity
across partition halves within a core:
  core c runs chart type c//4 (0=plain, 1=constrained) for batch slice
  (c%4)*64 : (c%4+1)*64.  Row r of the slice lives on partition pair
  (r, r+64): partition r computes EVEN chart positions, partition r+64
  ODD positions.  Both partitions keep a full replica of the row's
  chart; the high replica is stored shifted by one position (-128
  elements) so that a single instruction over all 128 partitions uses
  identical access patterns for both halves.

Chart layout (low replica): C[pos*128 + w] = cell(start=pos, width=w).
High replica: H[x] = C[x + 128].  At level l (L=128-l positions,
N=l split points, cnt=ceil(L/2) positions per parity):
    ls[i, n] = C[(2i+d)*128 + n]            -> local  256*i + n
    rs[i, n] = C[(2i+d+n+1)*128 + l-1-n]    -> local  256*i + 127*n + 127+l
    best[i] -> cell(2i+d, l)                -> local  256*i + l
with d=0 (low) / d=1 (high): the same offsets in both replicas.

Per-level cross-parity exchange: two PE matmuls with a 64x64 identity
swap the freshly written column between partition halves via PSUM, and
one ScalarE copy writes the other parity's values into each replica at
local offset 128+l (stride 256).  Out-of-range lanes only ever write to
storage slots with pos+width >= 128, which no valid access ever reads.
"""

import sys

if "/opt/trn_rl_repo" not in sys.path:
    sys.path.insert(0, "/opt/trn_rl_repo")

import numpy as np

LENGTH = 128
BATCH = 256
MARGIN = 1.0
BONUS = 1000.0
NCELLS = LENGTH * (LENGTH + 1) // 2  # 8256
NCONSTR = 8
N_CORES = 8
ROWS = 64        # batch rows per core (each on a partition pair)
CSTRIDE = 128    # chart elements per position
PSTRIDE = 256    # free-dim stride between same-parity positions
CH_FREE = LENGTH * CSTRIDE + 2 * CSTRIDE  # chart replica + guard pad


def _offsets():
    off = np.zeros(LENGTH, dtype=np.int64)
    for lvl in range(1, LENGTH):
        off[lvl] = off[lvl - 1] + (LENGTH - (lvl - 1))
    return off


OFF = _offsets()


def _score_offsets():
    soff, acc = {}, 0
    for level in range(1, LENGTH):
        soff[level] = acc
        acc += (LENGTH - level) * level
    return soff, acc


SOFF, TOTAL = _score_offsets()  # TOTAL = 349504


def _half_offsets():
    hoff, acc = {}, 0
    for level in range(1, LENGTH):
        L = LENGTH - level
        cnt = (L + 1) // 2
        hoff[level] = acc
        acc += cnt * level
    return hoff, acc


HOFF, TOTAL_HALF = _half_offsets()  # TOTAL_HALF = 176800

_COMPILED = None


def _build_program(repeat=1):
    from concourse import bacc, bass, mybir
    from concourse import tile

    nc = bacc.Bacc("TRN2", target_bir_lowering=False, debug=False,
                   num_devices=N_CORES)
    scores_ext = nc.dram_tensor("scores", [2 * ROWS, TOTAL_HALF],
                                mybir.dt.float32, kind="ExternalInput")
    roots_ext = nc.dram_tensor("roots", [ROWS, 1], mybir.dt.float32,
                               kind="ExternalOutput")

    # [128, 64] with a 64x64 identity in each partition half, so both
    # swap matmuls have lhsT.base_partition() == rhs.base_partition().
    ident_np = np.zeros((2 * ROWS, ROWS), np.float32)
    ident_np[np.arange(ROWS), np.arange(ROWS)] = 1.0
    ident_np[ROWS + np.arange(ROWS), np.arange(ROWS)] = 1.0
    ident_dram = nc.inline_tensor(ident_np, name="ident_c")

    f32 = mybir.dt.float32
    ADD = mybir.AluOpType.add
    MAX = mybir.AluOpType.max

    with tile.TileContext(nc) as tc:
        with (
            tc.tile_pool(name="persist", bufs=1) as persist,
            tc.tile_pool(name="xs", bufs=6) as xpool,
            tc.tile_pool(name="tmp", bufs=1) as tpool,
            tc.tile_pool(name="tmp2", bufs=1) as upool,
            tc.tile_pool(name="psum", bufs=2, space="PSUM") as ppool,
        ):
            C = persist.tile([2 * ROWS, CH_FREE], f32)
            ident = persist.tile([2 * ROWS, ROWS], f32, name="ident")

            # One-time init: zero the whole chart area (keeps the guard
            # pad and dead slots finite) and load the identity blocks.
            nc.vector.memset(C[:], 0.0)
            nc.sync.dma_start(out=ident[:], in_=ident_dram[:, :])

            c_full = C[:]
            c_part = c_full.ap[0]          # [partition_stride, 128]

            def c_view(offset, dims, p0=0, np_=2 * ROWS):
                part = [c_part[0], np_]
                return bass.AP(tensor=c_full.tensor,
                               offset=c_full.offset + p0 * c_part[0]
                               + offset,
                               ap=[part] + dims)

            for _rep in range(repeat):
                # width-0 cells = 0 for both replicas
                nc.vector.memset(c_view(0, [[CSTRIDE, LENGTH]]), 0.0)

                for l in range(1, LENGTH):
                    L = LENGTH - l
                    N = l
                    cnt = (L + 1) // 2
                    E2 = cnt * N

                    x_t = xpool.tile([2 * ROWS, E2], f32, tag="x")
                    nc.sync.dma_start(
                        out=x_t[:],
                        in_=scores_ext[:, HOFF[l]:HOFF[l] + E2])

                    ls = c_view(0, [[PSTRIDE, cnt], [1, N]])
                    rs = c_view(127 + l, [[PSTRIDE, cnt], [127, N]])

                    t_t = tpool.tile([2 * ROWS, E2], f32, tag="t")
                    tf = t_t[:]
                    t3 = bass.AP(tensor=tf.tensor, offset=tf.offset,
                                 ap=[tf.ap[0], [N, cnt], [1, N]])
                    nc.vector.tensor_tensor(out=t3, in0=ls, in1=rs, op=ADD)

                    u_t = upool.tile([2 * ROWS, E2], f32, tag="u")
                    nc.vector.tensor_tensor(out=u_t[:], in0=t_t[:],
                                            in1=x_t[:], op=ADD)
                    uf = u_t[:]
                    u3 = bass.AP(tensor=uf.tensor, offset=uf.offset,
                                 ap=[uf.ap[0], [N, cnt], [1, N]])

                    # best -> own-parity column slot (offset l, stride 256)
                    nc.vector.tensor_reduce(
                        out=c_view(l, [[PSTRIDE, cnt]]), in_=u3,
                        axis=mybir.AxisListType.X, op=MAX)

                    # cross-parity exchange via PE + PSUM
                    ps = ppool.tile([2 * ROWS, cnt], f32, tag="ps")
                    # low half <- odd bests (high partitions, col l)
                    nc.tensor.matmul(
                        out=ps[0:ROWS, :],
                        lhsT=ident[ROWS:2 * ROWS, :],
                        rhs=c_view(l, [[PSTRIDE, cnt]], p0=ROWS, np_=ROWS),
                        start=True, stop=True)
                    # high half <- even bests shifted by one position
                    nc.tensor.matmul(
                        out=ps[ROWS:2 * ROWS, :],
                        lhsT=ident[0:ROWS, :],
                        rhs=c_view(l + PSTRIDE, [[PSTRIDE, cnt]], p0=0,
                                   np_=ROWS),
                        start=True, stop=True)
                    # single 128-partition fill of the other parity
                    nc.scalar.copy(out=c_view(128 + l, [[PSTRIDE, cnt]]),
                                   in_=ps[:, :])

            # root = cell(pos=0, width=127) = low replica local 127
            nc.sync.dma_start(out=roots_ext[:], in_=C[0:ROWS, 127:128])

    nc.compile()
    return nc


def _get_compiled():
    global _COMPILED
    if _COMPILED is None:
        _COMPILED = _build_program()
    return _COMPILED


def _perm_scores(src):
    """[B, TOTAL] -> [B, 2, TOTAL_HALF]: even-position / odd-position
    per-level chunks, odd half zero-padded to cnt rows per level."""
    B = src.shape[0]
    out = np.zeros((B, 2, TOTAL_HALF), np.float32)
    for l in range(1, LENGTH):
        L = LENGTH - l
        N = l
        cnt = (L + 1) // 2
        sl = src[:, SOFF[l]:SOFF[l] + L * N].reshape(B, L, N)
        h = HOFF[l]
        out[:, 0, h:h + cnt * N] = sl[:, 0::2].reshape(B, cnt * N)
        no = L // 2
        out[:, 1, h:h + no * N] = sl[:, 1::2].reshape(B, no * N)
    return out


def make_inmaps(np_inputs):
    scores = np.asarray(np_inputs["scores"], dtype=np.float32)
    constraint_pos = np.asarray(np_inputs["constraint_pos"], dtype=np.int32)
    B = scores.shape[0]
    assert B == BATCH and scores.shape[1] == TOTAL

    # Constrained-chart scores: fold the +BONUS of cell (w, p) into every
    # split entry x[l=w][p, n] of that cell (exact up to 1 ulp vs adding
    # it after the reduce).  Set semantics (duplicate constraints count
    # once), matching `chart.at[rows, pos].set(BONUS)` in the reference.
    folded = scores.copy()
    bonus = np.float32(BONUS)
    for b in range(B):
        for f in set(int(v) for v in constraint_pos[b]):
            w = int(np.searchsorted(OFF, f, side="right")) - 1
            p = f - int(OFF[w])
            s = SOFF[w] + p * w
            folded[b, s:s + w] += bonus

    perm_plain = _perm_scores(scores)
    perm_fold = _perm_scores(folded)

    in_maps = []
    for c in range(N_CORES):
        q = c % 4
        sl = slice(q * ROWS, (q + 1) * ROWS)
        src = perm_plain if c < 4 else perm_fold
        blk = src[sl]  # [ROWS, 2, TOTAL_HALF]
        arr = np.concatenate([blk[:, 0], blk[:, 1]], axis=0)
        in_maps.append({"scores": np.ascontiguousarray(arr)})
    return in_maps


def kernel(scores, constraint_pos, trace=False):
    from concourse.bass_utils import run_bass_kernel_spmd

    in_maps = make_inmaps({"scores": scores,
                           "constraint_pos": constraint_pos})

    nc = _get_compiled()
    res = run_bass_kernel_spmd(nc, in_maps, list(range(N_CORES)),
                               trace=trace)

    pred = np.concatenate([res.results[q]["roots"][:, 0] for q in range(4)])
    constr_root = np.concatenate(
        [res.results[4 + q]["roots"][:, 0] for q in range(4)])

    pred = pred.astype(np.float32)
    constr = (constr_root - np.float32(BONUS * NCONSTR)).astype(np.float32)

    mask = (np.abs(pred - constr) >= np.float32(0.001)).astype(np.float32)
    hinge = np.maximum(np.float32(MARGIN) + pred - constr,
                       np.float32(0.0)) * mask
    msum = mask.sum(dtype=np.float32)
    hsum = hinge.sum(dtype=np.float32)
    if msum > np.float32(0.1):
        out = hsum / np.maximum(msum, np.float32(1.0))
    else:
        out = hsum
    result = np.asarray(out, dtype=np.float32)
    if trace:
        return result, res
    return result
